# revision 8
# baseline (speedup 1.0000x reference)
"""DeepseekV3 MLA decode attention (B=32, H=128, q_len=1, T=4096) on 8 trn2 NeuronCores.

Strategy: batch-parallel over the 8 cores (4 batches/core). Per core, the full
absorbed-MLA decode runs on device:
  - absorb:   q_lat[b,h,c] = q_nope[b,h,:] @ w_ukv[h]          (PE, col-tiled 4 heads/bank)
  - scores:   s[h,t] = q_lat[h,:] @ ckv_T + q_pe_rot[h,:] @ kpe_T   (PE, fp32)
  - softmax:  max/exp/sum on ACT+DVE (exp fused with denominator accumulation)
  - value:    x[h,c] = probs @ ckv                              (PE, probs transposed on PE)
  - out-proj: out[b,h,v] = x[b,h,:] @ w_v[h].T                  (PE, col-tiled)
Host prep is layout-only: RoPE of the single query token, cache transpose to
[c,t] for the scores matmul, weight reshapes. All matmul FLOPs stay on device.
"""

import functools

import numpy as np

B, H, NOPE, ROPE, V, LORA = 32, 128, 128, 64, 128, 512
CACHE_LEN, START_POS = 4096, 4095
T = START_POS + 1            # 4096 keys after cache append
CR = LORA + ROPE             # 576 rows of the transposed key matrix
SCALE = float((NOPE + ROPE) ** -0.5)
N_CORES = 8
BL = B // N_CORES            # 4 batches per core
NT = T // 512                # 8 score tiles of 512
NTC = T // 128               # 32 value chunks of 128
NKC = LORA // 128            # 4 latent c-chunks of 128


def _interleave_to_half(x):
    *lead, d = x.shape
    return x.reshape(*lead, d // 2, 2).swapaxes(-1, -2).reshape(*lead, d)


def _rotate_half(x):
    d = x.shape[-1]
    return np.concatenate([-x[..., d // 2:], x[..., :d // 2]], axis=-1)


def _host_rope(q_pe, k_pe, cos, sin, position_ids):
    kv_seq_len = 1
    c = cos[:kv_seq_len][position_ids][:, None]  # [B,1,1,64]
    s = sin[:kv_seq_len][position_ids][:, None]
    q = _interleave_to_half(q_pe)
    k = _interleave_to_half(k_pe)
    return q * c + _rotate_half(q) * s, k * c + _rotate_half(k) * s


@functools.lru_cache(maxsize=1)
def _build_program():
    import concourse.bacc as bacc
    import concourse.mybir as mybir
    import concourse.tile as tile
    from concourse.masks import make_identity

    f32 = mybir.dt.float32
    nc = bacc.Bacc("TRN2", target_bir_lowering=False, debug=False)

    # ---- DRAM I/O ----
    k_t = nc.dram_tensor("k_t", [BL, CR, T], f32, kind="ExternalInput").ap()
    v_c = nc.dram_tensor("v_c", [BL, CACHE_LEN, LORA], f32, kind="ExternalInput").ap()
    newkv = nc.dram_tensor("newkv", [BL, LORA], f32, kind="ExternalInput").ap()
    qn_t = nc.dram_tensor("qn_t", [NOPE, 32, 4, 32], f32, kind="ExternalInput").ap()
    qpe_t = nc.dram_tensor("qpe_t", [BL, ROPE, H], f32, kind="ExternalInput").ap()
    wu = nc.dram_tensor("wu", [32, NOPE, 4, LORA], f32, kind="ExternalInput").ap()
    wv_t = nc.dram_tensor("wv_t", [H, LORA, V], f32, kind="ExternalInput").ap()
    out_d = nc.dram_tensor("out_d", [BL, H, V], f32, kind="ExternalOutput").ap()

    from contextlib import ExitStack

    with tile.TileContext(nc) as tc, ExitStack() as st0:
        constp = st0.enter_context(tc.tile_pool(name="const", bufs=1))
        dramp = st0.enter_context(tc.tile_pool(name="dram", bufs=1, space="DRAM"))

        identity = constp.tile([128, 128], f32)
        make_identity(nc, identity)

        qn_sb = constp.tile([NOPE, 32, 4, 32], f32)
        nc.sync.dma_start(out=qn_sb, in_=qn_t)

        ql_d = dramp.tile([BL, LORA, H], f32)

        # ---------- Phase A: absorb q_nope @ w_ukv -> q_lat (DRAM round trip) ----------
        stA = ExitStack()
        wuP = stA.enter_context(tc.tile_pool(name="wuP", bufs=2))
        qlP = stA.enter_context(tc.tile_pool(name="qlP", bufs=1))
        psA = stA.enter_context(tc.tile_pool(name="psA", bufs=2, space="PSUM"))

        ql_sb = qlP.tile([128, LORA, 32], f32)  # part=(32a+b), free=(c,g)
        for g in range(32):
            wu_sb = wuP.tile([NOPE, 4, LORA], f32)
            nc.sync.dma_start(out=wu_sb, in_=wu[g])
            ps = psA.tile([128, LORA], f32)
            for a in range(4):
                # head h = 32a+g ; out rows 32a..32a+32 (batch-padded)
                nc.tensor.matmul(
                    ps[32 * a: 32 * a + 32, :],
                    qn_sb[:, g, a, :],
                    wu_sb[:, a, :],
                    start=True, stop=True,
                    tile_position=(0, 32 * a),
                )
            nc.vector.tensor_copy(ql_sb[:, :, g], ps)
        for a in range(4):
            nc.sync.dma_start(
                out=ql_d.rearrange("b c (a g) -> b c a g", a=4)[:, :, a, :],
                in_=ql_sb[32 * a: 32 * a + 4, :, :],
            )
        stA.close()

        # q lhsT tiles: [c-chunk(128), h] per batch + rope part
        lq = []
        lqro = []
        for b in range(BL):
            t_ = constp.tile([128, NKC, H], f32, tag=f"lq{b}", name=f"lq{b}")
            for k in range(NKC):
                nc.sync.dma_start(out=t_[:, k, :], in_=ql_d[b, 128 * k: 128 * (k + 1), :])
            lq.append(t_)
            r_ = constp.tile([ROPE, H], f32, tag=f"lqro{b}", name=f"lqro{b}")
            nc.sync.dma_start(out=r_, in_=qpe_t[b])
            lqro.append(r_)

        # ---------- Phase B: attention per batch ----------
        stB = ExitStack()
        ktP = stB.enter_context(tc.tile_pool(name="ktP", bufs=2))
        ktroP = stB.enter_context(tc.tile_pool(name="ktroP", bufs=2))
        vtP = stB.enter_context(tc.tile_pool(name="vtP", bufs=3))
        scP = stB.enter_context(tc.tile_pool(name="scP", bufs=1))
        prP = stB.enter_context(tc.tile_pool(name="prP", bufs=2))
        stP = stB.enter_context(tc.tile_pool(name="stP", bufs=2))
        pTP = stB.enter_context(tc.tile_pool(name="pTP", bufs=2))
        xP = stB.enter_context(tc.tile_pool(name="xP", bufs=1))
        xtP = stB.enter_context(tc.tile_pool(name="xtP", bufs=1))
        psS = stB.enter_context(tc.tile_pool(name="psS", bufs=2, space="PSUM"))
        psT = stB.enter_context(tc.tile_pool(name="psT", bufs=2, space="PSUM"))
        psX = stB.enter_context(tc.tile_pool(name="psX", bufs=1, space="PSUM"))
        psXT = stB.enter_context(tc.tile_pool(name="psXT", bufs=1, space="PSUM"))

        x_sb = [xP.tile([H, LORA], f32, tag=f"x{b}", name=f"x{b}") for b in range(BL)]
        xT_sb = [xtP.tile([128, BL, H], f32, tag=f"xt{k}", name=f"xt{k}") for k in range(NKC)]

        def attention_batch(b):
            scores = scP.tile([H, T], f32)
            mx8 = stP.tile([128, NT], f32, tag="mx8")
            for j in range(NT):
                kt = ktP.tile([128, NKC, 512], f32)
                nc.sync.dma_start(
                    out=kt,
                    in_=k_t[b, 0:LORA, 512 * j: 512 * (j + 1)].rearrange(
                        "(k p) t -> p k t", p=128
                    ),
                )
                ktro = ktroP.tile([ROPE, 512], f32)
                nc.sync.dma_start(
                    out=ktro, in_=k_t[b, LORA:CR, 512 * j: 512 * (j + 1)]
                )
                ps = psS.tile([H, 512], f32)
                for k in range(NKC):
                    nc.tensor.matmul(
                        ps, lq[b][:, k, :], kt[:, k, :],
                        start=(k == 0), stop=False,
                    )
                nc.tensor.matmul(ps, lqro[b], ktro, start=False, stop=True)
                nc.scalar.copy(scores[:, 512 * j: 512 * (j + 1)], ps)
                nc.vector.tensor_reduce(
                    mx8[:, j: j + 1], scores[:, 512 * j: 512 * (j + 1)],
                    axis=mybir.AxisListType.X, op=mybir.AluOpType.max,
                )
            mx = stP.tile([128, 1], f32, tag="mx")
            nc.vector.tensor_reduce(
                mx, mx8, axis=mybir.AxisListType.X, op=mybir.AluOpType.max
            )
            nbias = stP.tile([128, 1], f32, tag="nbias")
            nc.scalar.mul(nbias, mx, -SCALE)

            probs = prP.tile([H, T], f32)
            sums = stP.tile([128, NT], f32, tag="sums")
            for j in range(NT):
                nc.scalar.activation(
                    probs[:, 512 * j: 512 * (j + 1)],
                    scores[:, 512 * j: 512 * (j + 1)],
                    mybir.ActivationFunctionType.Exp,
                    bias=nbias, scale=SCALE,
                    accum_out=sums[:, j: j + 1],
                )
            ssum = stP.tile([128, 1], f32, tag="ssum")
            nc.vector.tensor_reduce(
                ssum, sums, axis=mybir.AxisListType.X, op=mybir.AluOpType.add
            )
            rs = stP.tile([128, 1], f32, tag="rs")
            nc.vector.reciprocal(rs, ssum)

            xps = psX.tile([H, LORA], f32)
            for j2 in range(NTC):
                pT_ps = psT.tile([128, 128], f32)
                nc.tensor.transpose(
                    pT_ps, probs[:, 128 * j2: 128 * (j2 + 1)], identity
                )
                pT = pTP.tile([128, 128], f32)
                nc.vector.tensor_copy(pT, pT_ps)
                vt = vtP.tile([128, LORA], f32)
                if j2 < NTC - 1:
                    nc.sync.dma_start(
                        out=vt, in_=v_c[b, 128 * j2: 128 * (j2 + 1), :]
                    )
                else:
                    nc.sync.dma_start(
                        out=vt[0:127, :], in_=v_c[b, 128 * j2: 128 * j2 + 127, :]
                    )
                    nc.sync.dma_start(out=vt[127:128, :], in_=newkv[b: b + 1, :])
                nc.tensor.matmul(
                    xps, pT, vt, start=(j2 == 0), stop=(j2 == NTC - 1)
                )
            nc.vector.tensor_scalar_mul(x_sb[b], xps, rs)

            # xT chunks for the output projection: [c,(b,h)]
            for k in range(NKC):
                xt_ps = psXT.tile([128, H], f32)
                nc.tensor.transpose(
                    xt_ps, x_sb[b][:, 128 * k: 128 * (k + 1)], identity
                )
                nc.vector.tensor_copy(xT_sb[k][:, b, :], xt_ps)

        for b in range(BL):
            attention_batch(b)

        # ---------- Phase D: out-proj ----------
        wvP = stB.enter_context(tc.tile_pool(name="wvP", bufs=2))
        outP = stB.enter_context(tc.tile_pool(name="outP", bufs=1))
        psO = stB.enter_context(tc.tile_pool(name="psO", bufs=2, space="PSUM"))

        out_sb = outP.tile([128, 32, V], f32)  # part=(32a+b), free=(m,v); h=4m+a
        for o in range(16):  # octets of heads
            wv_sb = wvP.tile([128, 8, NKC, V], f32)
            nc.sync.dma_start(
                out=wv_sb,
                in_=wv_t[8 * o: 8 * (o + 1)].rearrange(
                    "h (k p) v -> p h k v", p=128
                ),
            )
            for mm in range(2):  # two groups of 4 heads per octet
                pso = psO.tile([128, V], f32)
                m = 2 * o + mm
                for a in range(4):
                    h = 4 * m + a
                    i = h - 8 * o
                    for k in range(NKC):
                        nc.tensor.matmul(
                            pso[32 * a: 32 * a + 4, :],
                            xT_sb[k][:, :, h],
                            wv_sb[:, i, k, :],
                            start=(k == 0), stop=(k == NKC - 1),
                            tile_position=(0, 32 * a),
                        )
                for a in range(4):
                    nc.vector.tensor_copy(
                        out_sb[32 * a: 32 * a + 4, m, :],
                        pso[32 * a: 32 * a + 4, :],
                    )
        for a in range(4):
            nc.sync.dma_start(
                out=out_d.rearrange("b (m a) v -> b m a v", a=4)[:, :, a, :],
                in_=out_sb[32 * a: 32 * a + 4, :, :],
            )
        stB.close()

    nc.compile()
    return nc


def _host_prep(q_nope, q_pe, compressed_kv, k_pe, position_ids, attention_mask,
               start_pos, ckv_cache, k_pe_cache, sin, cos, wkv_b):
    q_nope = np.asarray(q_nope, np.float32)
    q_pe = np.asarray(q_pe, np.float32)
    compressed_kv = np.asarray(compressed_kv, np.float32)
    k_pe = np.asarray(k_pe, np.float32)
    position_ids = np.asarray(position_ids)
    attention_mask = np.asarray(attention_mask)
    ckv_cache = np.asarray(ckv_cache, np.float32)
    k_pe_cache = np.asarray(k_pe_cache, np.float32)
    sin = np.asarray(sin, np.float32)
    cos = np.asarray(cos, np.float32)
    wkv_b = np.asarray(wkv_b, np.float32)

    assert int(start_pos) == START_POS, f"kernel compiled for start_pos={START_POS}"
    assert not np.any(attention_mask), "kernel assumes an all-zero attention mask"

    q_pe_rot, k_pe_rot = _host_rope(q_pe, k_pe, cos, sin, position_ids)
    k_pe_rot = k_pe_rot[:, 0, 0, :]                      # [B, 64]

    w = wkv_b.reshape(H, NOPE + V, LORA)
    w_ukv = w[:, :NOPE, :]                               # [H, 128, 512]
    # wu[g, d, a, c] = w_ukv[32a+g, d, c]
    wu = np.ascontiguousarray(
        w_ukv.reshape(4, 32, NOPE, LORA).transpose(1, 2, 0, 3))
    # wv_t[h, c, v] = w_v[h, v, c]^T
    wv_t = np.ascontiguousarray(w[:, NOPE:, :].transpose(0, 2, 1))

    # transposed keys: [B, 576, 4096]
    kT = np.empty((B, CR, T), np.float32)
    kT[:, :LORA, :START_POS] = ckv_cache[:, :START_POS].transpose(0, 2, 1)
    kT[:, LORA:, :START_POS] = k_pe_cache[:, :START_POS].transpose(0, 2, 1)
    kT[:, :LORA, START_POS] = compressed_kv[:, 0]
    kT[:, LORA:, START_POS] = k_pe_rot

    in_maps = []
    for c in range(N_CORES):
        bs = slice(c * BL, (c + 1) * BL)
        # qn_t[d, g, a, b] = q_nope[bs[b], 32a+g, 0, d], zero-padded to b=32
        qn = np.zeros((NOPE, 32, 4, 32), np.float32)
        qn[:, :, :, :BL] = (
            q_nope[bs, :, 0, :]                          # [4, 128, 128] (b,h,d)
            .reshape(BL, 4, 32, NOPE)                    # (b, a, g, d)
            .transpose(3, 2, 1, 0)                       # (d, g, a, b)
        )
        in_maps.append({
            "k_t": np.ascontiguousarray(kT[bs]),
            "v_c": ckv_cache[bs],
            "newkv": compressed_kv[bs, 0, :],
            "qn_t": qn,
            "qpe_t": np.ascontiguousarray(q_pe_rot[bs, :, 0, :].transpose(0, 2, 1)),
            "wu": wu,
            "wv_t": wv_t,
        })
    return in_maps


_EXEC_CACHE = {}


def _get_executor(nc):
    """jit-once PJRT executor for the 8-core SPMD program (mirrors
    concourse.bass2jax.run_bass_via_pjrt, but reusable across calls)."""
    key = id(nc)
    if key in _EXEC_CACHE:
        return _EXEC_CACHE[key]

    import jax
    import concourse.mybir as mybir
    from concourse.bass2jax import (
        _bass_exec_p, install_neuronx_cc_hook, partition_id_tensor,
    )
    from jax.sharding import Mesh, PartitionSpec
    try:
        from jax.experimental.shard_map import shard_map
    except ImportError:  # newer jax
        from jax.shard_map import shard_map

    install_neuronx_cc_hook()
    assert nc.dbg_addr is None
    partition_name = nc.partition_id_tensor.name if nc.partition_id_tensor else None

    in_names, out_names, out_avals = [], [], []
    for alloc in nc.m.functions[0].allocations:
        if not isinstance(alloc, mybir.MemoryLocationSet):
            continue
        name = alloc.memorylocations[0].name
        if alloc.kind == "ExternalInput":
            if name != partition_name:
                in_names.append(name)
        elif alloc.kind == "ExternalOutput":
            out_names.append(name)
            out_avals.append(jax.core.ShapedArray(
                tuple(alloc.tensor_shape), mybir.dt.np(alloc.dtype)))
    n_params = len(in_names)
    all_names = in_names + out_names
    if partition_name is not None:
        all_names = all_names + [partition_name]

    def _body(*args):
        operands = list(args)
        if partition_name is not None:
            operands.append(partition_id_tensor())
        outs = _bass_exec_p.bind(
            *operands,
            out_avals=tuple(out_avals),
            in_names=tuple(all_names),
            out_names=tuple(out_names),
            lowering_input_output_aliases=(),
            sim_require_finite=True,
            sim_require_nnan=True,
            nc=nc,
        )
        return tuple(outs)

    devices = jax.devices()[:N_CORES]
    mesh = Mesh(np.asarray(devices), ("core",))
    donate = tuple(range(n_params, n_params + len(out_names)))
    fn = jax.jit(
        shard_map(
            _body, mesh=mesh,
            in_specs=(PartitionSpec("core"),) * (n_params + len(out_names)),
            out_specs=(PartitionSpec("core"),) * len(out_names),
            check_rep=False,
        ),
        donate_argnums=donate, keep_unused=True,
    )
    entry = (fn, mesh, in_names, out_names, out_avals, n_params)
    _EXEC_CACHE[key] = entry
    return entry


def _execute(nc, in_maps, device_args=None):
    """Run once; returns (list-of-per-core output dicts, device_args)."""
    import jax

    fn, mesh, in_names, out_names, out_avals, n_params = _get_executor(nc)
    if device_args is None:
        from jax.sharding import NamedSharding, PartitionSpec
        sh = NamedSharding(mesh, PartitionSpec("core"))
        concat_in = [
            np.concatenate([np.asarray(m[name]) for m in in_maps], axis=0)
            for name in in_names
        ]
        device_args = [jax.device_put(a, sh) for a in concat_in]
    zeros = [np.zeros((N_CORES * av.shape[0], *av.shape[1:]), av.dtype)
             for av in out_avals]
    outs = fn(*device_args, *zeros)
    outs = [np.asarray(o) for o in outs]
    results = [
        {name: outs[i].reshape(N_CORES, *out_avals[i].shape)[c]
         for i, name in enumerate(out_names)}
        for c in range(N_CORES)
    ]
    return results, device_args


def kernel(**inputs) -> np.ndarray:
    nc = _build_program()
    in_maps = _host_prep(**inputs)
    results, _ = _execute(nc, in_maps)
    out = np.empty((B, 1, H, V), np.float32)
    for c in range(N_CORES):
        out[c * BL:(c + 1) * BL, 0] = results[c]["out_d"]
    return out


# revision 12
# speedup vs baseline: 68.3862x; 68.3862x over previous
"""DeepseekV3 MLA decode attention (B=32, H=128, q_len=1, T=4096) on 8 trn2 NeuronCores.

Strategy: batch-parallel over the 8 cores (4 batches/core). Per core, the full
absorbed-MLA decode runs on device:
  - absorb:   q_lat[b,h,c] = q_nope[b,h,:] @ w_ukv[h]          (PE, col-tiled 4 heads/bank)
  - scores:   s[h,t] = q_lat[h,:] @ ckv_T + q_pe_rot[h,:] @ kpe_T   (PE, fp32)
  - softmax:  max/exp/sum on ACT+DVE (exp fused with denominator accumulation)
  - value:    x[h,c] = probs @ ckv                              (PE, probs transposed on PE)
  - out-proj: out[b,h,v] = x[b,h,:] @ w_v[h].T                  (PE, col-tiled)
Host prep is layout-only: RoPE of the single query token, cache transpose to
[c,t] for the scores matmul, weight reshapes. All matmul FLOPs stay on device.
"""

import functools

import numpy as np

B, H, NOPE, ROPE, V, LORA = 32, 128, 128, 64, 128, 512
CACHE_LEN, START_POS = 4096, 4095
T = START_POS + 1            # 4096 keys after cache append
CR = LORA + ROPE             # 576 rows of the transposed key matrix
SCALE = float((NOPE + ROPE) ** -0.5)
N_CORES = 8
BL = B // N_CORES            # 4 batches per core
NT = T // 512                # 8 score tiles of 512
NTC = T // 128               # 32 value chunks of 128
NKC = LORA // 128            # 4 latent c-chunks of 128


def _interleave_to_half(x):
    *lead, d = x.shape
    return x.reshape(*lead, d // 2, 2).swapaxes(-1, -2).reshape(*lead, d)


def _rotate_half(x):
    d = x.shape[-1]
    return np.concatenate([-x[..., d // 2:], x[..., :d // 2]], axis=-1)


def _host_rope(q_pe, k_pe, cos, sin, position_ids):
    kv_seq_len = 1
    c = cos[:kv_seq_len][position_ids][:, None]  # [B,1,1,64]
    s = sin[:kv_seq_len][position_ids][:, None]
    q = _interleave_to_half(q_pe)
    k = _interleave_to_half(k_pe)
    return q * c + _rotate_half(q) * s, k * c + _rotate_half(k) * s


@functools.lru_cache(maxsize=4)
def _build_program(reps=1, bf16=True):
    import concourse.bacc as bacc
    import concourse.mybir as mybir
    import concourse.tile as tile
    from concourse.masks import make_identity

    f32 = mybir.dt.float32
    dk = mybir.dt.float16 if bf16 else f32
    nc = bacc.Bacc("TRN2", target_bir_lowering=False, debug=False)

    # ---- DRAM I/O ----
    k_t = nc.dram_tensor("k_t", [BL, CR, T], dk, kind="ExternalInput").ap()
    v_c = nc.dram_tensor("v_c", [BL, CACHE_LEN, LORA], dk, kind="ExternalInput").ap()
    newkv = nc.dram_tensor("newkv", [BL, LORA], dk, kind="ExternalInput").ap()
    qn_t = nc.dram_tensor("qn_t", [NOPE, 32, 4, 32], dk, kind="ExternalInput").ap()
    qpe_t = nc.dram_tensor("qpe_t", [BL, ROPE, H], dk, kind="ExternalInput").ap()
    wu = nc.dram_tensor("wu", [32, NOPE, 4, LORA], dk, kind="ExternalInput").ap()
    wv_t = nc.dram_tensor("wv_t", [H, LORA, V], dk, kind="ExternalInput").ap()
    out_d = nc.dram_tensor("out_d", [BL, H, V], f32, kind="ExternalOutput").ap()

    from contextlib import ExitStack

    with tile.TileContext(nc) as tc, ExitStack() as st0:
        constp = st0.enter_context(tc.tile_pool(name="const", bufs=1))
        dramp = st0.enter_context(tc.tile_pool(name="dram", bufs=1, space="DRAM"))

        identity = constp.tile([128, 128], dk)
        make_identity(nc, identity)

        qn_sb = constp.tile([NOPE, 32, 4, 32], dk)
        nc.sync.dma_start(out=qn_sb, in_=qn_t)

        ql_d = dramp.tile([BL, LORA, H], dk)

        for _rep in range(reps):
            _emit_body(nc, tc, mybir, f32, dk, _rep,
                       identity, qn_sb, k_t, v_c, newkv, qpe_t, wu, wv_t,
                       ql_d, out_d)

    nc.compile()
    return nc


def _emit_body(nc, tc, mybir, f32, dk, rep,
               identity, qn_sb, k_t, v_c, newkv, qpe_t, wu, wv_t, ql_d, out_d):
    from contextlib import ExitStack

    def pname(s):
        return f"{s}_r{rep}"

    with ExitStack() as stC:
        constp = stC.enter_context(tc.tile_pool(name=pname("perrep"), bufs=1))

        # ---------- Phase A: absorb q_nope @ w_ukv -> q_lat (DRAM round trip) ----------
        stA = ExitStack()
        wuP = stA.enter_context(tc.tile_pool(name=pname("wuP"), bufs=2))
        qlP = stA.enter_context(tc.tile_pool(name=pname("qlP"), bufs=1))
        psA = stA.enter_context(tc.tile_pool(name=pname("psA"), bufs=2, space="PSUM"))

        ql_sb = qlP.tile([128, LORA, 32], dk)  # part=(32a+b), free=(c,g)
        for g in range(32):
            wu_sb = wuP.tile([NOPE, 4, LORA], dk)
            nc.sync.dma_start(out=wu_sb, in_=wu[g])
            ps = psA.tile([128, LORA], f32)
            for a in range(4):
                # head h = 32a+g ; out rows 32a..32a+32 (batch-padded)
                nc.tensor.matmul(
                    ps[32 * a: 32 * a + 32, :],
                    qn_sb[:, g, a, :],
                    wu_sb[:, a, :],
                    start=True, stop=True,
                    tile_position=(0, 32 * a),
                )
            nc.vector.tensor_copy(ql_sb[:, :, g], ps)
        for a in range(4):
            nc.sync.dma_start(
                out=ql_d.rearrange("b c (a g) -> b c a g", a=4)[:, :, a, :],
                in_=ql_sb[32 * a: 32 * a + 4, :, :],
            )
        stA.close()

        # q lhsT tiles: [c-chunk(128), h] per batch + rope part
        lq = []
        lqro = []
        for b in range(BL):
            t_ = constp.tile([128, NKC, H], dk, tag=f"lq{b}", name=f"lq{b}_r{rep}")
            for k in range(NKC):
                nc.sync.dma_start(out=t_[:, k, :], in_=ql_d[b, 128 * k: 128 * (k + 1), :])
            lq.append(t_)
            r_ = constp.tile([ROPE, H], dk, tag=f"lqro{b}", name=f"lqro{b}_r{rep}")
            nc.sync.dma_start(out=r_, in_=qpe_t[b])
            lqro.append(r_)

        # ---------- Phase B: attention per batch ----------
        stB = ExitStack()
        ktP = stB.enter_context(tc.tile_pool(name=pname("ktP"), bufs=2))
        ktroP = stB.enter_context(tc.tile_pool(name=pname("ktroP"), bufs=2))
        vtP = stB.enter_context(tc.tile_pool(name=pname("vtP"), bufs=3))
        scP = stB.enter_context(tc.tile_pool(name=pname("scP"), bufs=1))
        prP = stB.enter_context(tc.tile_pool(name=pname("prP"), bufs=2))
        stP = stB.enter_context(tc.tile_pool(name=pname("stP"), bufs=2))
        pTP = stB.enter_context(tc.tile_pool(name=pname("pTP"), bufs=2))
        xP = stB.enter_context(tc.tile_pool(name=pname("xP"), bufs=1))
        xtP = stB.enter_context(tc.tile_pool(name=pname("xtP"), bufs=1))
        psS = stB.enter_context(tc.tile_pool(name=pname("psS"), bufs=2, space="PSUM"))
        psT = stB.enter_context(tc.tile_pool(name=pname("psT"), bufs=2, space="PSUM"))
        psX = stB.enter_context(tc.tile_pool(name=pname("psX"), bufs=1, space="PSUM"))
        psXT = stB.enter_context(tc.tile_pool(name=pname("psXT"), bufs=1, space="PSUM"))

        x_sb = [xP.tile([H, LORA], dk, tag=f"x{b}", name=f"x{b}_r{rep}") for b in range(BL)]
        xT_sb = [xtP.tile([128, BL, H], dk, tag=f"xt{k}", name=f"xt{k}_r{rep}") for k in range(NKC)]

        def attention_batch(b):
            scores = scP.tile([H, T], f32)
            mx8 = stP.tile([128, NT], f32, tag="mx8")
            for j in range(NT):
                kt = ktP.tile([128, NKC, 512], dk)
                nc.sync.dma_start(
                    out=kt,
                    in_=k_t[b, 0:LORA, 512 * j: 512 * (j + 1)].rearrange(
                        "(k p) t -> p k t", p=128
                    ),
                )
                ktro = ktroP.tile([ROPE, 512], dk)
                nc.sync.dma_start(
                    out=ktro, in_=k_t[b, LORA:CR, 512 * j: 512 * (j + 1)]
                )
                ps = psS.tile([H, 512], f32)
                for k in range(NKC):
                    nc.tensor.matmul(
                        ps, lq[b][:, k, :], kt[:, k, :],
                        start=(k == 0), stop=False,
                    )
                nc.tensor.matmul(ps, lqro[b], ktro, start=False, stop=True)
                nc.scalar.copy(scores[:, 512 * j: 512 * (j + 1)], ps)
                nc.vector.tensor_reduce(
                    mx8[:, j: j + 1], scores[:, 512 * j: 512 * (j + 1)],
                    axis=mybir.AxisListType.X, op=mybir.AluOpType.max,
                )
            mx = stP.tile([128, 1], f32, tag="mx")
            nc.vector.tensor_reduce(
                mx, mx8, axis=mybir.AxisListType.X, op=mybir.AluOpType.max
            )
            nbias = stP.tile([128, 1], f32, tag="nbias")
            nc.scalar.mul(nbias, mx, -SCALE)

            probs = prP.tile([H, T], dk)
            sums = stP.tile([128, NT], f32, tag="sums")
            for j in range(NT):
                nc.scalar.activation(
                    probs[:, 512 * j: 512 * (j + 1)],
                    scores[:, 512 * j: 512 * (j + 1)],
                    mybir.ActivationFunctionType.Exp,
                    bias=nbias, scale=SCALE,
                    accum_out=sums[:, j: j + 1],
                )
            ssum = stP.tile([128, 1], f32, tag="ssum")
            nc.vector.tensor_reduce(
                ssum, sums, axis=mybir.AxisListType.X, op=mybir.AluOpType.add
            )
            rs = stP.tile([128, 1], f32, tag="rs")
            nc.vector.reciprocal(rs, ssum)

            xps = psX.tile([H, LORA], f32)
            for j2 in range(NTC):
                pT_ps = psT.tile([128, 128], dk)
                nc.tensor.transpose(
                    pT_ps, probs[:, 128 * j2: 128 * (j2 + 1)], identity
                )
                pT = pTP.tile([128, 128], dk)
                nc.vector.tensor_copy(pT, pT_ps)
                vt = vtP.tile([128, LORA], dk)
                if j2 < NTC - 1:
                    nc.sync.dma_start(
                        out=vt, in_=v_c[b, 128 * j2: 128 * (j2 + 1), :]
                    )
                else:
                    nc.sync.dma_start(
                        out=vt[0:127, :], in_=v_c[b, 128 * j2: 128 * j2 + 127, :]
                    )
                    nc.sync.dma_start(out=vt[127:128, :], in_=newkv[b: b + 1, :])
                nc.tensor.matmul(
                    xps, pT, vt, start=(j2 == 0), stop=(j2 == NTC - 1)
                )
            nc.vector.tensor_scalar_mul(x_sb[b], xps, rs)

            # xT chunks for the output projection: [c,(b,h)]
            for k in range(NKC):
                xt_ps = psXT.tile([128, H], dk)
                nc.tensor.transpose(
                    xt_ps, x_sb[b][:, 128 * k: 128 * (k + 1)], identity
                )
                nc.vector.tensor_copy(xT_sb[k][:, b, :], xt_ps)

        for b in range(BL):
            attention_batch(b)

        # ---------- Phase D: out-proj ----------
        wvP = stB.enter_context(tc.tile_pool(name=pname("wvP"), bufs=2))
        outP = stB.enter_context(tc.tile_pool(name=pname("outP"), bufs=1))
        psO = stB.enter_context(tc.tile_pool(name=pname("psO"), bufs=2, space="PSUM"))

        out_sb = outP.tile([128, 32, V], f32)  # part=(32a+b), free=(m,v); h=4m+a
        for o in range(16):  # octets of heads
            wv_sb = wvP.tile([128, 8, NKC, V], dk)
            nc.sync.dma_start(
                out=wv_sb,
                in_=wv_t[8 * o: 8 * (o + 1)].rearrange(
                    "h (k p) v -> p h k v", p=128
                ),
            )
            for mm in range(2):  # two groups of 4 heads per octet
                pso = psO.tile([128, V], f32)
                m = 2 * o + mm
                for a in range(4):
                    h = 4 * m + a
                    i = h - 8 * o
                    for k in range(NKC):
                        nc.tensor.matmul(
                            pso[32 * a: 32 * a + 4, :],
                            xT_sb[k][:, :, h],
                            wv_sb[:, i, k, :],
                            start=(k == 0), stop=(k == NKC - 1),
                            tile_position=(0, 32 * a),
                        )
                for a in range(4):
                    nc.vector.tensor_copy(
                        out_sb[32 * a: 32 * a + 4, m, :],
                        pso[32 * a: 32 * a + 4, :],
                    )
        for a in range(4):
            nc.sync.dma_start(
                out=out_d.rearrange("b (m a) v -> b m a v", a=4)[:, :, a, :],
                in_=out_sb[32 * a: 32 * a + 4, :, :],
            )
        stB.close()


USE_BF16 = True


def _host_prep(q_nope, q_pe, compressed_kv, k_pe, position_ids, attention_mask,
               start_pos, ckv_cache, k_pe_cache, sin, cos, wkv_b, bf16=None):
    if bf16 is None:
        bf16 = USE_BF16
    dk = np.float16 if bf16 else np.float32
    q_nope = np.asarray(q_nope, np.float32)
    q_pe = np.asarray(q_pe, np.float32)
    compressed_kv = np.asarray(compressed_kv, np.float32)
    k_pe = np.asarray(k_pe, np.float32)
    position_ids = np.asarray(position_ids)
    attention_mask = np.asarray(attention_mask)
    ckv_cache = np.asarray(ckv_cache, np.float32)
    k_pe_cache = np.asarray(k_pe_cache, np.float32)
    sin = np.asarray(sin, np.float32)
    cos = np.asarray(cos, np.float32)
    wkv_b = np.asarray(wkv_b, np.float32)

    assert int(start_pos) == START_POS, f"kernel compiled for start_pos={START_POS}"
    assert not np.any(attention_mask), "kernel assumes an all-zero attention mask"

    q_pe_rot, k_pe_rot = _host_rope(q_pe, k_pe, cos, sin, position_ids)
    k_pe_rot = k_pe_rot[:, 0, 0, :]                      # [B, 64]

    w = wkv_b.reshape(H, NOPE + V, LORA)
    w_ukv = w[:, :NOPE, :]                               # [H, 128, 512]
    # wu[g, d, a, c] = w_ukv[32a+g, d, c]
    wu = np.ascontiguousarray(
        w_ukv.reshape(4, 32, NOPE, LORA).transpose(1, 2, 0, 3)).astype(dk)
    # wv_t[h, c, v] = w_v[h, v, c]^T
    wv_t = np.ascontiguousarray(w[:, NOPE:, :].transpose(0, 2, 1)).astype(dk)

    # transposed keys: [B, 576, 4096]
    kT = np.empty((B, CR, T), dk)
    kT[:, :LORA, :START_POS] = ckv_cache[:, :START_POS].transpose(0, 2, 1)
    kT[:, LORA:, :START_POS] = k_pe_cache[:, :START_POS].transpose(0, 2, 1)
    kT[:, :LORA, START_POS] = compressed_kv[:, 0]
    kT[:, LORA:, START_POS] = k_pe_rot

    in_maps = []
    for c in range(N_CORES):
        bs = slice(c * BL, (c + 1) * BL)
        # qn_t[d, g, a, b] = q_nope[bs[b], 32a+g, 0, d], zero-padded to b=32
        qn = np.zeros((NOPE, 32, 4, 32), dk)
        qn[:, :, :, :BL] = (
            q_nope[bs, :, 0, :]                          # [4, 128, 128] (b,h,d)
            .reshape(BL, 4, 32, NOPE)                    # (b, a, g, d)
            .transpose(3, 2, 1, 0)                       # (d, g, a, b)
        )
        in_maps.append({
            "k_t": np.ascontiguousarray(kT[bs]),
            "v_c": ckv_cache[bs].astype(dk) if bf16 else ckv_cache[bs],
            "newkv": compressed_kv[bs, 0, :].astype(dk),
            "qn_t": qn,
            "qpe_t": np.ascontiguousarray(
                q_pe_rot[bs, :, 0, :].transpose(0, 2, 1)).astype(dk),
            "wu": wu,
            "wv_t": wv_t,
        })
    return in_maps


_EXEC_CACHE = {}


def _get_executor(nc):
    """jit-once PJRT executor for the 8-core SPMD program (mirrors
    concourse.bass2jax.run_bass_via_pjrt, but reusable across calls)."""
    key = id(nc)
    if key in _EXEC_CACHE:
        return _EXEC_CACHE[key]

    import jax
    import concourse.mybir as mybir
    from concourse.bass2jax import (
        _bass_exec_p, install_neuronx_cc_hook, partition_id_tensor,
    )
    from jax.sharding import Mesh, PartitionSpec
    try:
        from jax.experimental.shard_map import shard_map
    except ImportError:  # newer jax
        from jax.shard_map import shard_map

    install_neuronx_cc_hook()
    assert nc.dbg_addr is None
    partition_name = nc.partition_id_tensor.name if nc.partition_id_tensor else None

    in_names, out_names, out_avals = [], [], []
    for alloc in nc.m.functions[0].allocations:
        if not isinstance(alloc, mybir.MemoryLocationSet):
            continue
        name = alloc.memorylocations[0].name
        if alloc.kind == "ExternalInput":
            if name != partition_name:
                in_names.append(name)
        elif alloc.kind == "ExternalOutput":
            out_names.append(name)
            out_avals.append(jax.core.ShapedArray(
                tuple(alloc.tensor_shape), mybir.dt.np(alloc.dtype)))
    n_params = len(in_names)
    all_names = in_names + out_names
    if partition_name is not None:
        all_names = all_names + [partition_name]

    def _body(*args):
        operands = list(args)
        if partition_name is not None:
            operands.append(partition_id_tensor())
        outs = _bass_exec_p.bind(
            *operands,
            out_avals=tuple(out_avals),
            in_names=tuple(all_names),
            out_names=tuple(out_names),
            lowering_input_output_aliases=(),
            sim_require_finite=True,
            sim_require_nnan=True,
            nc=nc,
        )
        return tuple(outs)

    devices = jax.devices()[:N_CORES]
    mesh = Mesh(np.asarray(devices), ("core",))
    donate = tuple(range(n_params, n_params + len(out_names)))
    fn = jax.jit(
        shard_map(
            _body, mesh=mesh,
            in_specs=(PartitionSpec("core"),) * (n_params + len(out_names)),
            out_specs=(PartitionSpec("core"),) * len(out_names),
            check_rep=False,
        ),
        donate_argnums=donate, keep_unused=True,
    )
    entry = (fn, mesh, in_names, out_names, out_avals, n_params)
    _EXEC_CACHE[key] = entry
    return entry


def _execute(nc, in_maps, device_args=None):
    """Run once; returns (list-of-per-core output dicts, device_args)."""
    import jax

    fn, mesh, in_names, out_names, out_avals, n_params = _get_executor(nc)
    if device_args is None:
        from jax.sharding import NamedSharding, PartitionSpec
        sh = NamedSharding(mesh, PartitionSpec("core"))
        concat_in = [
            np.concatenate([np.asarray(m[name]) for m in in_maps], axis=0)
            for name in in_names
        ]
        device_args = [jax.device_put(a, sh) for a in concat_in]
    zeros = [np.zeros((N_CORES * av.shape[0], *av.shape[1:]), av.dtype)
             for av in out_avals]
    outs = fn(*device_args, *zeros)
    outs = [np.asarray(o) for o in outs]
    results = [
        {name: outs[i].reshape(N_CORES, *out_avals[i].shape)[c]
         for i, name in enumerate(out_names)}
        for c in range(N_CORES)
    ]
    return results, device_args


def kernel(**inputs) -> np.ndarray:
    nc = _build_program(1, USE_BF16)
    in_maps = _host_prep(**inputs, bf16=USE_BF16)
    results, _ = _execute(nc, in_maps)
    out = np.empty((B, 1, H, V), np.float32)
    for c in range(N_CORES):
        out[c * BL:(c + 1) * BL, 0] = results[c]["out_d"]
    return out
